# revision 1
# baseline (speedup 1.0000x reference)
"""DUPLEX GAT on trn2 — kernel builder + host glue.

Design:
  - Nodes permuted into NW windows of 128 (degree-balanced), padded to N_pad.
  - Per core c the node tables are ROTATED so that core-local dst windows are
    rows [0, wpc*128) of its private g/er tables -> one SPMD program, all
    per-core variation lives in input data.
  - Phase A (per conv, both stacks): g = x @ (W @ T_bd) node-major table in
    DRAM (T_bd = per-head [attn_l | orth-complement] basis so el rides in g
    cols h*32), er table for own windows.
  - Phase B: per dst-window, per src-half group (int16 gather limit):
    dma_gather g rows by src; S / ST one-hot via iota compares; er broadcast
    to edges via ST matmul; p = exp(leakyrelu(el+er)); c = p*w;
    Z *= c (per head); scatter: num'T += Z_t.T @ S_t, den += p_t.T @ S_t in
    PSUM; epilogue: num'/den, un-transform by T_bd^-1 (+ mean over heads for
    layer 1), bias (+ReLU layer 0).
  - Softmax max-subtraction dropped (exact cancellation; logits are O(3)).
"""
import sys
sys.path.insert(0, '/opt/trn_rl_repo')
from dataclasses import dataclass

import numpy as np

import concourse.bass as bass
import concourse.bacc as bacc
import concourse.tile as tile
from concourse import mybir, library_config

F32 = mybir.dt.float32
I16 = mybir.dt.int16
P = 128
NEG = 0.2


def _patch_drain_split():
    """The installed walrus rejects >1 sem wait on the kernel-tail Drain;
    split the waits across a chain of drains."""
    import bass_rust
    from concourse.tile import ScopedClock

    def patched(self, tick_clock, wait_clock):
        nc = self.nc
        drain_inst = nc.sync.drain()
        wait_clock.add_sem_waits(
            drain_inst.ins, ScopedClock({None: tick_clock.global_clock}))
        si = drain_inst.ins.sync_info
        waits = list(si.on_wait) if si is not None else []
        if len(waits) > 1:
            si.on_wait = waits[:1]
            for i in range(1, len(waits)):
                d2 = nc.sync.drain()
                d2.ins.sync_info = bass_rust.SyncInfo(
                    on_wait=waits[i : i + 1], on_update=[])
        nc.all_engine_barrier()
        popped = nc._tile_sem_poison_stack.pop()
        assert popped is self._sem_poison
        nc.clear_and_free_semaphores(list(self.sems.allocated().values()))
        nc.all_engine_barrier()

    tile.TileContext._drain_and_barrier = patched


_patch_drain_split()


def _patch_loud_ncc():
    """Surface compile-hook exceptions (swallowed by the PJRT plugin)."""
    import traceback
    from concourse import bass2jax
    if getattr(bass2jax, "_loud_ncc", False):
        return
    bass2jax._loud_ncc = True
    orig = bass2jax.neuronx_cc_hook

    def logged(*a, **k):
        try:
            return orig(*a, **k)
        except BaseException:
            with open("/tmp/ncc_hook_err.log", "a") as f:
                f.write(traceback.format_exc() + "\n")
            raise

    bass2jax.neuronx_cc_hook = logged


_patch_loud_ncc()


@dataclass
class Cfg:
    n_nodes: int = 50000
    n_edges: int = 800000
    n_cores: int = 8
    wpc: int = 49           # windows per core
    split: int = 32768      # int16 gather split (rows per table half A)
    heads: int = 4
    fdim: int = 32
    in_dim: int = 128
    gbatch: int = 0         # tiles per gather call (0 = whole group)
    bf16: bool = False      # bf16 scatter matmuls (S/Z/p), er path stays fp32

    @property
    def nw(self):
        return self.n_cores * self.wpc

    @property
    def n_pad(self):
        return self.nw * P


# ----------------------------------------------------------------- host prep

def balance_windows(deg, cfg):
    """Assign nodes (incl pad) to windows, balancing total degree via LPT."""
    import heapq
    n_pad, nw = cfg.n_pad, cfg.nw
    degp = np.zeros(n_pad, np.int64)
    degp[: len(deg)] = deg
    order = np.argsort(-degp, kind="stable")
    heap = [(0, w, 0) for w in range(nw)]  # (load, window, count)
    heapq.heapify(heap)
    win_of = np.empty(n_pad, np.int32)
    slot_of = np.empty(n_pad, np.int32)
    pending = []  # windows that reached 128
    for node in order:
        load, w, cnt = heapq.heappop(heap)
        win_of[node] = w
        slot_of[node] = cnt
        cnt += 1
        if cnt < P:
            heapq.heappush(heap, (load + degp[node], w, cnt))
    new_id = win_of.astype(np.int64) * P + slot_of
    return new_id  # (n_pad,) position of each (padded) node id


def wrap_idx(idx):
    """(n,) -> (128, n//16) int16 wrapped layout for dma_gather."""
    n = len(idx)
    blk = np.asarray(idx, np.int16).reshape(n // 16, 16).T
    return np.tile(blk, (8, 1))


def prep_graph(src, dst, cfg):
    """Build per-core gather/scatter tables. Returns dict."""
    c = cfg
    deg = np.bincount(dst, minlength=c.n_nodes)
    new_id = balance_windows(deg, c)              # old(+pad) -> new position
    node_at = np.full(c.n_pad, -1, np.int64)      # new position -> old id
    node_at[new_id] = np.arange(c.n_pad)

    src_n = new_id[src]
    dst_n = new_id[dst]
    order = np.argsort(dst_n, kind="stable")
    src_s, dst_s = src_n[order], dst_n[order]
    win_s = dst_s // P
    # edge ranges per window
    bounds = np.searchsorted(win_s, np.arange(c.nw + 1))

    # per-core rotated row of a (new-space) node position, per core
    # rot_row(core, pos) = ((pos//P - core*wpc) % nw)*P + pos%P
    def rot_rows(core, pos):
        return ((pos // P - core * c.wpc) % c.nw) * P + pos % P

    # first pass: per-window group sizes in each core's rotation
    # group A iff rot_row < split. rotation differs per core *only* via the
    # window part of src; sizes therefore differ per core.
    TA = TB = 0
    grp_masks = {}
    for core in range(c.n_cores):
        for wl in range(c.wpc):
            g = core * c.wpc + wl
            lo, hi = bounds[g], bounds[g + 1]
            rr = rot_rows(core, src_s[lo:hi])
            mA = rr < c.split
            grp_masks[(core, wl)] = (lo, hi, rr, mA)
            nA = int(mA.sum())
            nB = int((hi - lo) - nA)
            TA = max(TA, -(-nA // P))
            TB = max(TB, -(-nB // P))
    TA = max(TA, 1)
    TB = max(TB, 1)
    T = TA + TB

    nco, wpc = c.n_cores, c.wpc
    idxA = np.zeros((nco, wpc, P, TA * 8), np.int16)
    idxB = np.zeros((nco, wpc, P, TB * 8), np.int16)
    dstmb = np.full((nco, wpc, P, T), -1000.0, np.float32)
    dstrow = np.full((nco, wpc, 1, T * P), -1000.0, np.float32)
    epos = np.full((nco, wpc, T * P), -1, np.int64)  # orig edge slot (sorted order)

    for core in range(nco):
        for wl in range(wpc):
            lo, hi, rr, mA = grp_masks[(core, wl)]
            iA = np.where(mA)[0]
            iB = np.where(~mA)[0]
            la = np.zeros(TA * P, np.int64)   # gather rows group A (pad->0)
            lb = np.zeros(TB * P, np.int64)
            la[: len(iA)] = rr[iA]
            lb[: len(iB)] = rr[iB] - c.split
            idxA[core, wl] = wrap_idx(la)
            idxB[core, wl] = wrap_idx(lb)
            dmb = np.full(T * P, -1000.0, np.float32)
            dmb[: len(iA)] = (dst_s[lo:hi][iA] % P).astype(np.float32)
            dmb[TA * P : TA * P + len(iB)] = (dst_s[lo:hi][iB] % P).astype(np.float32)
            dstmb[core, wl] = dmb.reshape(T, P).T
            dstrow[core, wl, 0] = dmb
            ep = np.full(T * P, -1, np.int64)
            ep[: len(iA)] = lo + iA
            ep[TA * P : TA * P + len(iB)] = lo + iB
            epos[core, wl] = ep

    return dict(
        new_id=new_id, node_at=node_at, order=order, TA=TA, TB=TB,
        idxA=idxA, idxB=idxB, dstmb=dstmb, dstrow=dstrow, epos=epos,
    )


def edge_w_tables(g, w_edge, cfg):
    """Per-core per-window padded edge-weight tables (nc, wpc, P, T)."""
    ws = np.asarray(w_edge)[g["order"]]
    ep = g["epos"]
    out = np.where(ep >= 0, ws[np.clip(ep, 0, None)], 0.0).astype(np.float32)
    n, w, TP = out.shape
    T = TP // P
    return out.reshape(n, w, T, P).transpose(0, 1, 3, 2).copy()


def prep_conv(W, al, ar, b, mean_heads, cfg):
    """Derived weights for one conv. Returns dict of f32 arrays."""
    H, F = cfg.heads, cfg.fdim
    W = np.asarray(W, np.float64)
    al = np.asarray(al, np.float64)
    ar = np.asarray(ar, np.float64)
    b = np.asarray(b, np.float64)
    T_bd = np.zeros((H * F, H * F))
    for h in range(H):
        a = al[h]
        M = np.concatenate([a[:, None], np.eye(F)[:, : F - 1]], 1)
        Q, _ = np.linalg.qr(M)
        blk = np.concatenate([a[:, None], Q[:, 1:]], 1)
        T_bd[h * F : (h + 1) * F, h * F : (h + 1) * F] = blk
    Tinv = np.linalg.inv(T_bd)
    R_bd = np.zeros((H * F, H))
    for h in range(H):
        R_bd[h * F : (h + 1) * F, h] = ar[h]
    out = dict(
        Wg=np.concatenate([W @ T_bd, W @ R_bd], 1).astype(np.float32),
    )
    if mean_heads:
        Mm = np.zeros((H * F, F))
        for h in range(H):
            Mm[h * F : (h + 1) * F] = np.eye(F) / H
        out["Tinv"] = (Tinv @ Mm).astype(np.float32)          # (128, 32)
        out["bcol"] = (b.reshape(H, F).mean(0))[:, None].astype(np.float32)
    else:
        out["Tinv"] = Tinv.astype(np.float32)                  # (128, 128)
        out["bcol"] = b[:, None].astype(np.float32)            # (128, 1)
    return out


def consts_np(cfg):
    H, F = cfg.heads, cfg.fdim
    iotab = np.tile(np.arange(P, dtype=np.float32)[None, :], (P, 1))
    iotac = np.arange(P, dtype=np.float32)[:, None]
    ones1 = np.ones((1, P), np.float32)
    H2 = np.zeros((2, 2 * H, H * F), np.float32)  # [stack, 8, 128]
    for s in range(2):
        for h in range(H):
            H2[s, s * H + h, h * F : (h + 1) * F] = 1.0
    return dict(iotab=iotab, iotac=iotac, ones1=ones1, H2am=H2[0], H2ph=H2[1])


def rotate_tiles(x_tiled, core, cfg):
    """x_tiled (nw,128,128) -> rotated copy for `core`."""
    rot = np.roll(np.arange(cfg.nw), -core * cfg.wpc)
    return np.ascontiguousarray(x_tiled[rot])


def to_xT_tiled(x, g, cfg):
    """x (n_nodes, D) -> permuted transposed tiles (nw, D, 128) f32."""
    n_pad = cfg.n_pad
    D = x.shape[1]
    xp = np.zeros((n_pad, D), np.float32)
    real = g["node_at"] >= 0
    idx = g["node_at"][real]
    keep = idx < cfg.n_nodes
    xp[np.where(real)[0][keep]] = np.asarray(x, np.float32)[idx[keep]]
    return np.ascontiguousarray(xp.reshape(cfg.nw, P, D).transpose(0, 2, 1))


# ------------------------------------------------------------ layer program

def build_layer(cfg, TA, TB, last, debug_level=3):
    """One Bass program: both stacks of one GAT layer. Returns (nc, io)."""
    c = cfg
    T = TA + TB
    HF = c.heads * c.fdim            # 128
    OC = c.fdim if last else HF      # output channels per node
    nc = bacc.Bacc("TRN2", target_bir_lowering=False, debug=False)

    inp = {}

    def dram_in(name, shape, dt=F32):
        inp[name] = nc.dram_tensor(name, list(shape), dt, kind="ExternalInput")
        return inp[name]

    xT = {s: dram_in(f"xT_{s}", (c.nw, c.in_dim, P)) for s in ("am", "ph")}
    Wg = {s: dram_in(f"Wg_{s}", (c.in_dim, HF + c.heads)) for s in ("am", "ph")}
    Tinv = {s: dram_in(f"Tinv_{s}", (HF, OC)) for s in ("am", "ph")}
    bcol = {s: dram_in(f"bcol_{s}", (OC, 1)) for s in ("am", "ph")}
    H2 = {s: dram_in(f"H2_{s}", (2 * c.heads, HF)) for s in ("am", "ph")}
    iotab_d = dram_in("iotab", (P, P))
    iotac_d = dram_in("iotac", (P, 1))
    ones_d = dram_in("ones1", (1, P))
    idxA_d = dram_in("idxA", (c.wpc, P, TA * 8), I16)
    idxB_d = dram_in("idxB", (c.wpc, P, TB * 8), I16)
    dstmb_d = dram_in("dstmb", (c.wpc, P, T))
    dstrow_d = dram_in("dstrow", (c.wpc, 1, T * P))
    wtab = {s: dram_in(f"wtab_{s}", (c.wpc, P, T)) for s in ("am", "ph")}

    out_t = {
        s: nc.dram_tensor(f"out_{s}", [c.wpc, OC, P], F32, kind="ExternalOutput")
        for s in ("am", "ph")
    }

    stacks = ("am", "ph")

    with tile.TileContext(nc) as tc:
        with (
            tc.tile_pool(name="dram", bufs=1, space="DRAM") as dpool,
            tc.tile_pool(name="const", bufs=1) as cpool,
        ):
            gtab = {s: dpool.tile([c.n_pad, HF], F32, name=f"gtab{s}", tag=f"gtab{s}") for s in stacks}
            ertab = dpool.tile([c.wpc * P, 2 * c.heads], F32, name="ertab")

            ct = {}
            for nm, hd, sh in [
                ("iotab", iotab_d, (P, P)), ("iotac", iotac_d, (P, 1)),
                ("ones", ones_d, (1, P)),
            ]:
                ct[nm] = cpool.tile(list(sh), F32, name=f"ct_{nm}")
                nc.sync.dma_start(ct[nm][:], hd[:])
            for s in stacks:
                for nm, hd in [("Wg", Wg[s]), ("Tinv", Tinv[s]),
                               ("bcol", bcol[s]), ("H2", H2[s])]:
                    t = cpool.tile(list(hd.shape), F32, name=f"ct_{nm}_{s}")
                    nc.sync.dma_start(t[:], hd[:])
                    ct[(nm, s)] = t

            # ---------------- phase A: g / er tables ----------------
            CH = next(k for k in (7, 4, 2, 1) if c.wpc % k == 0 and c.nw % k == 0)
            with (
                tc.tile_pool(name="pa_x", bufs=3) as pax,
                tc.tile_pool(name="pa_g", bufs=3) as pag,
                tc.tile_pool(name="pa_ps", bufs=4, space="PSUM") as paps,
                tc.tile_pool(name="pa_eps", bufs=2, space="PSUM") as paeps,
            ):
                for w0 in range(0, c.nw, CH):
                    do_er = w0 < c.wpc
                    er_sb = (pag.tile([P, CH, 2 * c.heads], F32, name="er_sb",
                                      tag="ersb") if do_er else None)
                    for si, s in enumerate(stacks):
                        xw = pax.tile([c.in_dim, CH, P], F32, name="xw", tag="x")
                        nc.sync.dma_start(
                            xw[:], xT[s][w0 : w0 + CH].rearrange("w d p -> d w p"))
                        g_sb = pag.tile([P, CH, HF], F32, name="g_sb", tag="gsb")
                        for k in range(CH):
                            g_ps = paps.tile([P, HF + c.heads], F32,
                                             name="g_ps", tag="g")
                            nc.tensor.matmul(g_ps[:], xw[:, k, :],
                                             ct[("Wg", s)][:],
                                             start=True, stop=True)
                            if do_er:
                                nc.vector.tensor_copy(
                                    er_sb[:, k, si * c.heads : (si + 1) * c.heads],
                                    g_ps[:, HF : HF + c.heads])
                            nc.any.tensor_copy(g_sb[:, k, :], g_ps[:, 0:HF])
                        nc.sync.dma_start(
                            gtab[s][w0 * P : (w0 + CH) * P, :]
                            .rearrange("(w p) d -> p w d", p=P), g_sb[:])
                    if do_er:
                        nc.sync.dma_start(
                            ertab[w0 * P : (w0 + CH) * P, :]
                            .rearrange("(w p) d -> p w d", p=P), er_sb[:])

            # ---------------- phase B: windows ----------------
            with (
                tc.tile_pool(name="pb_meta", bufs=2) as pbm,
                tc.tile_pool(name="pb_z", bufs=2) as pbz,
                tc.tile_pool(name="pb_s", bufs=2) as pbs,
                tc.tile_pool(name="pb_small", bufs=2) as pbsm,
                tc.tile_pool(name="pb_epi", bufs=2) as pbepi,
                tc.tile_pool(name="ps_acc", bufs=1, space="PSUM") as psacc,
                tc.tile_pool(name="ps_er", bufs=2, space="PSUM") as pser,
                tc.tile_pool(name="ps_bc", bufs=1, space="PSUM") as psbc,
                tc.tile_pool(name="ps_epi", bufs=2, space="PSUM") as psepi,
            ):
                nidx_reg = {} if debug_level >= 2 else None

                def _reg(n):
                    if n not in nidx_reg:
                        nidx_reg[n] = nc.gpsimd.to_reg(n)
                    return nidx_reg[n]
                for wl in range(c.wpc if debug_level >= 1 else 0):
                    idx_t = {}
                    for nm, hd, tt in [("A", idxA_d, TA), ("B", idxB_d, TB)]:
                        it = pbm.tile([P, tt * 8], I16, name=f"idx{nm}", tag=f"idx{nm}")
                        nc.sync.dma_start(it[:], hd[wl])
                        idx_t[nm] = it
                    dmb = pbm.tile([P, T], F32, name="dmb", tag="dmb")
                    nc.sync.dma_start(dmb[:], dstmb_d[wl])
                    drow = pbm.tile([1, T * P], F32, name="drow", tag="drow")
                    nc.sync.dma_start(drow[:], dstrow_d[wl])
                    wt = {}
                    for s in stacks:
                        wt[s] = pbm.tile([P, T], F32, name=f"wt{s}", tag=f"wt{s}")
                        nc.sync.dma_start(wt[s][:], wtab[s][wl])
                    erw = pbm.tile([P, 2 * c.heads], F32, name="erw", tag="erw")
                    nc.sync.dma_start(erw[:], ertab[wl * P : (wl + 1) * P, :])

                    # gathers
                    Z = {}
                    for s in stacks:
                        for gn, tt, base in [("A", TA, 0), ("B", TB, c.split)]:
                            zt = pbz.tile([P, tt, HF], F32, name=f"z{gn}{s}", tag=f"z{gn}{s}")
                            src_ap = (gtab[s][0 : c.split, :] if gn == "A"
                                      else gtab[s][c.split : c.n_pad, :])
                            if debug_level >= 2:
                                gb = c.gbatch or tt
                                for off in range(0, tt, gb):
                                    nb = min(gb, tt - off)
                                    nc.gpsimd.dma_gather(
                                        out_ap=zt[:, off : off + nb, :],
                                        in_ap=src_ap,
                                        idxs_ap=idx_t[gn][:, off * 8 : (off + nb) * 8],
                                        num_idxs=nb * P, num_idxs_reg=_reg(nb * P),
                                        elem_size=HF)
                            else:
                                nc.vector.memset(zt[:], 0.01)
                            Z[(s, gn)] = zt

                    # S one-hots (edges x nodes), per group
                    SDT = mybir.dt.bfloat16 if c.bf16 else F32
                    S = {}
                    for gn, tt, off in [("A", TA, 0), ("B", TB, TA)]:
                        st_ = pbs.tile([P, tt, P], SDT, name=f"S{gn}", tag=f"S{gn}")
                        nc.vector.tensor_tensor(
                            out=st_[:],
                            in0=ct["iotab"][:].unsqueeze(1).broadcast_to([P, tt, P]),
                            in1=dmb[:, off : off + tt].unsqueeze(2)
                                .broadcast_to([P, tt, P]),
                            op=mybir.AluOpType.is_equal)
                        S[gn] = st_

                    # ST (nodes x edges) via K=1 bcast matmul + compare
                    ST = {}
                    for gn, tt, off in [("A", TA, 0), ("B", TB, TA)]:
                        stt = pbs.tile([P, tt * P], F32, name=f"ST{gn}", tag=f"ST{gn}")
                        ncols_total = tt * P
                        ch0 = 0
                        while ch0 < ncols_total:
                            cw = min(512, ncols_total - ch0)
                            bc = psbc.tile([P, 512], F32, name="bc", tag="bc")
                            nc.tensor.matmul(
                                bc[:, 0:cw], ct["ones"][:],
                                drow[:, off * P + ch0 : off * P + ch0 + cw],
                                start=True, stop=True)
                            nc.vector.tensor_tensor(
                                out=stt[:, ch0 : ch0 + cw], in0=bc[:, 0:cw],
                                in1=ct["iotac"][:].broadcast_to([P, cw]),
                                op=mybir.AluOpType.is_equal)
                            ch0 += cw
                        ST[gn] = stt

                    # er per edge: er_ps[:, t, :] = ST_t.T @ er_win  (edges x 8)
                    er_ps = pser.tile([P, T, 2 * c.heads], F32, name="er_ps", tag="erps")
                    for t in range(T):
                        gn, tl = ("A", t) if t < TA else ("B", t - TA)
                        nc.tensor.matmul(
                            er_ps[:, t, :],
                            ST[gn][:, tl * P : (tl + 1) * P], erw[:],
                            start=True, stop=True)

                    # p = exp(leakyrelu(el + er)); c = p * w; Z *= c (per head)
                    pboth = {}
                    for gn, tt, off in [("A", TA, 0), ("B", TB, TA)]:
                        tb = pbsm.tile([P, tt, 2 * c.heads], F32, name=f"t{gn}", tag=f"t{gn}")
                        for si, s in enumerate(stacks):
                            el = (Z[(s, gn)][:]
                                  .rearrange("p t (h f) -> p t h f", h=c.heads)
                                  [:, :, :, 0:1].squeeze(3))
                            nc.vector.tensor_tensor(
                                out=tb[:, :, si * c.heads : (si + 1) * c.heads],
                                in0=el,
                                in1=er_ps[:, off : off + tt,
                                          si * c.heads : (si + 1) * c.heads],
                                op=mybir.AluOpType.add)
                        nc.vector.scalar_tensor_tensor(
                            out=tb[:], in0=tb[:], scalar=NEG, in1=tb[:],
                            op0=mybir.AluOpType.mult, op1=mybir.AluOpType.max)
                        pb_ = pbsm.tile([P, tt, 2 * c.heads], SDT, name=f"p{gn}", tag=f"p{gn}")
                        nc.scalar.activation(pb_[:], tb[:],
                                             mybir.ActivationFunctionType.Exp)
                        cb = pbsm.tile([P, tt, 2 * c.heads], F32, name=f"c{gn}", tag=f"c{gn}")
                        for si, s in enumerate(stacks):
                            nc.vector.tensor_tensor(
                                out=cb[:, :, si * c.heads : (si + 1) * c.heads],
                                in0=pb_[:, :, si * c.heads : (si + 1) * c.heads],
                                in1=wt[s][:, off : off + tt].unsqueeze(2)
                                    .broadcast_to([P, tt, c.heads]),
                                op=mybir.AluOpType.mult)
                            if c.bf16:
                                zb = pbz.tile([P, tt, HF], SDT,
                                              name=f"zb{gn}{s}", tag=f"zb{gn}{s}")
                                zdst = zb
                            else:
                                zdst = Z[(s, gn)]
                            nc.vector.tensor_tensor(
                                out=zdst[:].rearrange(
                                    "p t (h f) -> p t h f", h=c.heads),
                                in0=Z[(s, gn)][:].rearrange(
                                    "p t (h f) -> p t h f", h=c.heads),
                                in1=cb[:, :, si * c.heads : (si + 1) * c.heads]
                                    .unsqueeze(3)
                                    .broadcast_to([P, tt, c.heads, c.fdim]),
                                op=mybir.AluOpType.mult)
                            if c.bf16:
                                Z[(s, gn)] = zb
                        pboth[gn] = pb_

                    # scatter
                    num_ps = {s: psacc.tile([HF, P], F32, name=f"num{s}", tag=f"num{s}")
                              for s in stacks}
                    den_ps = psacc.tile([2 * c.heads, P], F32, name="den_ps", tag="den")
                    for t in range(T):
                        gn, tl = ("A", t) if t < TA else ("B", t - TA)
                        for s in stacks:
                            nc.tensor.matmul(
                                num_ps[s][:], Z[(s, gn)][:, tl, :],
                                S[gn][:, tl, :],
                                start=(t == 0), stop=(t == T - 1))
                        nc.tensor.matmul(
                            den_ps[:], pboth[gn][:, tl, :], S[gn][:, tl, :],
                            start=(t == 0), stop=(t == T - 1))

                    # epilogue
                    denm = pbepi.tile([2 * c.heads, P], F32, name="denm", tag="denm")
                    nc.vector.tensor_scalar(
                        out=denm[:], in0=den_ps[:], scalar1=1e-9, scalar2=None,
                        op0=mybir.AluOpType.max)
                    rec = pbepi.tile([2 * c.heads, P], F32, name="rec", tag="rec")
                    nc.vector.reciprocal(rec[:], denm[:])
                    for s in stacks:
                        dex = psepi.tile([P, P], F32, name="dex", tag="epi")
                        nc.tensor.matmul(dex[:], ct[("H2", s)][:], rec[:],
                                         start=True, stop=True)
                        dex_sb = pbepi.tile([P, P], F32, name="dex_sb", tag="dex_sb")
                        nc.any.tensor_copy(dex_sb[:], dex[:])
                        sca = pbepi.tile([HF, P], F32, name="sca", tag="sca")
                        nc.vector.tensor_tensor(out=sca[:], in0=num_ps[s][:],
                                                in1=dex_sb[:],
                                                op=mybir.AluOpType.mult)
                        hps = psepi.tile([OC, P], F32, name="hps", tag="epi")
                        nc.tensor.matmul(hps[:], ct[("Tinv", s)][:], sca[:],
                                         start=True, stop=True)
                        hsb = pbepi.tile([OC, P], F32, name="hsb", tag="hsb")
                        nc.scalar.activation(
                            hsb[:], hps[:],
                            (mybir.ActivationFunctionType.Identity if last
                             else mybir.ActivationFunctionType.Relu),
                            bias=ct[("bcol", s)][:], scale=1.0)
                        nc.sync.dma_start(out_t[s][wl], hsb[:])

    return _finish(nc)


def _finish(nc):
    nc.compile()
    return nc


# ------------------------------------------------------------ full pipeline

def make_in_maps(cfg, g, cc, xT_am_full, xT_ph_full, w_am_tab, w_ph_tab,
                 conv_am, conv_ph):
    """Build per-core input dicts for one layer launch."""
    maps = []
    for core in range(cfg.n_cores):
        m = dict(
            xT_am=rotate_tiles(xT_am_full, core, cfg),
            xT_ph=rotate_tiles(xT_ph_full, core, cfg),
            Wg_am=conv_am["Wg"], Tinv_am=conv_am["Tinv"],
            bcol_am=conv_am["bcol"], Wg_ph=conv_ph["Wg"],
            Tinv_ph=conv_ph["Tinv"], bcol_ph=conv_ph["bcol"],
            H2_am=cc["H2am"], H2_ph=cc["H2ph"], iotab=cc["iotab"],
            iotac=cc["iotac"], ones1=cc["ones1"],
            idxA=g["idxA"][core], idxB=g["idxB"][core],
            dstmb=g["dstmb"][core], dstrow=g["dstrow"][core],
            wtab_am=w_am_tab[core], wtab_ph=w_ph_tab[core],
        )
        maps.append(m)
    return maps


def assemble(outs, cfg, oc):
    """per-core out (wpc, OC, P) list -> (nw*P, OC) permuted-node-major."""
    full = np.concatenate([o.reshape(cfg.wpc, oc, P) for o in outs], 0)
    return full.transpose(0, 2, 1).reshape(cfg.n_pad, oc)


def run_pipeline(inputs, cfg, runner):
    """runner(nc, in_maps) -> list of per-core {name: np.ndarray} outputs."""
    g = prep_graph(np.asarray(inputs["src"]), np.asarray(inputs["dst"]), cfg)
    cc = consts_np(cfg)
    w_am = edge_w_tables(g, inputs["am_exist"], cfg)
    w_ph = edge_w_tables(g, inputs["exist"], cfg)

    conv0a = prep_conv(inputs["W0a"], inputs["al0a"], inputs["ar0a"],
                       inputs["b0a"], False, cfg)
    conv0p = prep_conv(inputs["W0p"], inputs["al0p"], inputs["ar0p"],
                       inputs["b0p"], False, cfg)
    conv1a = prep_conv(inputs["W1a"], inputs["al1a"], inputs["ar1a"],
                       inputs["b1a"], True, cfg)
    conv1p = prep_conv(inputs["W1p"], inputs["al1p"], inputs["ar1p"],
                       inputs["b1p"], True, cfg)

    xT_am = to_xT_tiled(np.asarray(inputs["x_am"]), g, cfg)
    xT_ph = to_xT_tiled(np.asarray(inputs["x_ph"]), g, cfg)

    nc0 = build_layer(cfg, g["TA"], g["TB"], last=False)
    maps0 = make_in_maps(cfg, g, cc, xT_am, xT_ph, w_am, w_ph, conv0a, conv0p)
    outs0 = runner(nc0, maps0)

    hT_am = np.concatenate([o["out_am"] for o in outs0], 0)  # (nw,128,128)
    hT_ph = np.concatenate([o["out_ph"] for o in outs0], 0)

    nc1 = build_layer(cfg, g["TA"], g["TB"], last=True)
    maps1 = make_in_maps(cfg, g, cc, hT_am, hT_ph, w_am, w_ph, conv1a, conv1p)
    outs1 = runner(nc1, maps1)

    oam = assemble([o["out_am"] for o in outs1], cfg, cfg.fdim)
    oph = assemble([o["out_ph"] for o in outs1], cfg, cfg.fdim)
    res_am = np.zeros((cfg.n_nodes, cfg.fdim), np.float32)
    res_ph = np.zeros((cfg.n_nodes, cfg.fdim), np.float32)
    nid = g["new_id"][: cfg.n_nodes]
    res_am[:] = oam[nid]
    res_ph[:] = oph[nid]
    return res_am, res_ph


# ------------------------------------------------------------ timed runner

def run_layer_timed(nc, in_maps, n_cores, repeats=3):
    """Execute with device-resident inputs; returns (results, times)."""
    import time as _time
    import jax
    from jax.sharding import Mesh, PartitionSpec, NamedSharding
    from jax.experimental.shard_map import shard_map
    from concourse import bass2jax

    bass2jax.install_neuronx_cc_hook()
    part_name = (nc.partition_id_tensor.name
                 if nc.partition_id_tensor is not None else None)
    in_names, out_names, out_avals, zero_outs = [], [], [], []
    for alloc in nc.m.functions[0].allocations:
        if not isinstance(alloc, mybir.MemoryLocationSet):
            continue
        name = alloc.memorylocations[0].name
        if alloc.kind == "ExternalInput":
            if name != part_name:
                in_names.append(name)
        elif alloc.kind == "ExternalOutput":
            out_names.append(name)
            shape = tuple(alloc.tensor_shape)
            dtype = mybir.dt.np(alloc.dtype)
            out_avals.append(jax.core.ShapedArray(shape, dtype))
            zero_outs.append(np.zeros(shape, dtype))
    n_params = len(in_names)
    all_in = list(in_names + out_names)
    if part_name is not None:
        all_in.append(part_name)

    def _body(*args):
        operands = list(args)
        if part_name is not None:
            operands.append(bass2jax.partition_id_tensor())
        outs = bass2jax._bass_exec_p.bind(
            *operands, out_avals=tuple(out_avals), in_names=tuple(all_in),
            out_names=tuple(out_names), lowering_input_output_aliases=(),
            sim_require_finite=True, sim_require_nnan=True, nc=nc)
        return tuple(outs)

    devices = jax.devices()[:n_cores]
    mesh = Mesh(np.asarray(devices), ("core",))
    spec = PartitionSpec("core")
    nin = n_params + len(out_names)
    f = jax.jit(shard_map(_body, mesh=mesh, in_specs=(spec,) * nin,
                          out_specs=(spec,) * len(out_names), check_rep=False))
    concat_in = [np.concatenate([np.asarray(m[nm]) for m in in_maps], 0)
                 for nm in in_names]
    concat_zeros = [np.zeros((n_cores * z.shape[0], *z.shape[1:]), z.dtype)
                    for z in zero_outs]
    sh = NamedSharding(mesh, spec)
    dev_in = [jax.device_put(a, sh) for a in concat_in]
    dev_zero = [jax.device_put(a, sh) for a in concat_zeros]
    outs = f(*dev_in, *dev_zero)
    jax.block_until_ready(outs)
    ts = []
    for _ in range(repeats):
        t0 = _time.perf_counter()
        o2 = f(*dev_in, *dev_zero)
        jax.block_until_ready(o2)
        ts.append(_time.perf_counter() - t0)
    res = []
    for c in range(n_cores):
        res.append({nm: np.asarray(outs[i]).reshape(n_cores, *out_avals[i].shape)[c]
                    for i, nm in enumerate(out_names)})
    return res, ts


def baseline_overhead(n_cores, repeats=5):
    """Dispatch+network floor: trivial 8-core kernel timed the same way."""
    nc = bacc.Bacc("TRN2", target_bir_lowering=False, debug=False)
    x = nc.dram_tensor("x", [P, P], F32, kind="ExternalInput")
    y = nc.dram_tensor("y", [P, P], F32, kind="ExternalOutput")
    with tile.TileContext(nc) as tc:
        with tc.tile_pool(name="p", bufs=1) as p:
            t = p.tile([P, P], F32)
            nc.sync.dma_start(t[:], x[:])
            nc.scalar.mul(t[:], t[:], 2.0)
            nc.sync.dma_start(y[:], t[:])
    nc.compile()
    maps = [{"x": np.zeros((P, P), np.float32)} for _ in range(n_cores)]
    _, ts = run_layer_timed(nc, maps, n_cores, repeats=repeats)
    return min(ts)


# ------------------------------------------------------------ kernel entry

_PERF = {"exec_ns": 0.0, "launch_info": []}


def _hw_runner(cfg, measure):
    from concourse.bass_utils import run_bass_kernel_spmd

    def run(nc, in_maps):
        if measure:
            res, ts = run_layer_timed(nc, in_maps, cfg.n_cores, repeats=3)
            _PERF["launch_info"].append(min(ts))
            return [{k: r[k].reshape(cfg.wpc, -1, P)
                     for k in ("out_am", "out_ph")} for r in res]
        res = run_bass_kernel_spmd(nc, in_maps,
                                   core_ids=list(range(cfg.n_cores)))
        return [{k: r[k].reshape(cfg.wpc, -1, P)
                 for k in ("out_am", "out_ph")} for r in res.results]
    return run


def kernel(**inputs):
    """Full DUPLEX-GAT forward on 8 trn2 cores. Returns (h_am, h_ph)."""
    import os
    cfg = Cfg(gbatch=int(os.environ.get("GAT_GBATCH", "4")),
              bf16=bool(int(os.environ.get("GAT_BF16", "0"))))
    measure = bool(int(os.environ.get("GAT_MEASURE", "0")))
    res_am, res_ph = run_pipeline(inputs, cfg, _hw_runner(cfg, measure))
    return res_am, res_ph



# revision 2
# speedup vs baseline: 20.9107x; 20.9107x over previous
"""DUPLEX GAT on trn2 — v2: bf16 phase-B, interleaved g table, node-major scatter.

Design (per core, SPMD over 8 cores; nodes permuted into nw=392 windows of 128,
rotated per core so own dst windows are rows [0, wpc*128) of its table):
  - Phase A: per window w, per stack s: g_ps = xw_s.T @ Wg_s where
    Wg_s = [W@T_bd | W@R_bd] (132 cols: ft in el-carrying basis + er(4)).
    ft cols -> interleaved bf16 DRAM table row [am(128) | ph(128)] (512B rows);
    er cols -> SBUF erw tile (own windows only).
  - Phase B per dst window: one dma_gather per index group (A: rows<32768 of
    the rotated table, B: rest) pulls [128, tt, 256] bf16 (both stacks per row).
    S one-hot (edges x nodes) built bf16 on DVE via iota==dmb.
    ST = PE transpose of S tiles (bf16 PSUM) -> Act copy to SBUF.
    er per edge: er_ps[:,t,:] = ST_t.T @ erw_w  (PSUM f32).
    tb = el (strided Z cols) + er; lrelu (DVE); p = exp (Act, bf16).
    cb = p * w_edge; Z *= cb (per 32-block).
    num_ps [128 nodes, 256] += S_t.T @ Z_t ; den_ps [128, 8] += S_t.T @ p_t.
    Epilogue: rec = 1/max(den,eps); sca = num*rec (bf16); per stack PE
    transpose -> [feat, nodes]; h = Tinv.T @ sca; bias + relu/identity; out.
"""
import sys
sys.path.insert(0, '/opt/trn_rl_repo')
from dataclasses import dataclass

import numpy as np
import ml_dtypes

import concourse.bass as bass
import concourse.bacc as bacc
import concourse.tile as tile
from concourse import mybir

F32 = mybir.dt.float32
BF16 = mybir.dt.bfloat16
I16 = mybir.dt.int16
P = 128
NEG = 0.2
BF = ml_dtypes.bfloat16


def _patch_drain_split():
    import bass_rust
    from concourse.tile import ScopedClock
    if getattr(tile.TileContext, "_drain_patched_v2", False):
        return

    def patched(self, tick_clock, wait_clock):
        nc = self.nc
        drain_inst = nc.sync.drain()
        wait_clock.add_sem_waits(
            drain_inst.ins, ScopedClock({None: tick_clock.global_clock}))
        si = drain_inst.ins.sync_info
        waits = list(si.on_wait) if si is not None else []
        if len(waits) > 1:
            si.on_wait = waits[:1]
            for i in range(1, len(waits)):
                d2 = nc.sync.drain()
                d2.ins.sync_info = bass_rust.SyncInfo(
                    on_wait=waits[i : i + 1], on_update=[])
        nc.all_engine_barrier()
        popped = nc._tile_sem_poison_stack.pop()
        assert popped is self._sem_poison
        nc.clear_and_free_semaphores(list(self.sems.allocated().values()))
        nc.all_engine_barrier()

    tile.TileContext._drain_and_barrier = patched
    tile.TileContext._drain_patched_v2 = True


_patch_drain_split()


def _patch_loud_ncc():
    import traceback
    from concourse import bass2jax
    if getattr(bass2jax, "_loud_ncc", False):
        return
    bass2jax._loud_ncc = True
    orig = bass2jax.neuronx_cc_hook

    def logged(*a, **k):
        try:
            return orig(*a, **k)
        except BaseException:
            with open("/tmp/ncc_hook_err.log", "a") as f:
                f.write(traceback.format_exc() + "\n")
            raise

    bass2jax.neuronx_cc_hook = logged


_patch_loud_ncc()


@dataclass
class Cfg:
    n_nodes: int = 50000
    n_edges: int = 800000
    n_cores: int = 8
    wpc: int = 49
    split: int = 32768
    heads: int = 4
    fdim: int = 32
    in_dim: int = 128
    gbatch: int = 8         # tiles per dma_gather call (HW desc-ring limit)

    @property
    def nw(self):
        return self.n_cores * self.wpc

    @property
    def n_pad(self):
        return self.nw * P


# ----------------------------------------------------------------- host prep

def balance_windows(deg, cfg):
    import heapq
    n_pad, nw = cfg.n_pad, cfg.nw
    degp = np.zeros(n_pad, np.int64)
    degp[: len(deg)] = deg
    order = np.argsort(-degp, kind="stable")
    heap = [(0, w, 0) for w in range(nw)]
    heapq.heapify(heap)
    win_of = np.empty(n_pad, np.int32)
    slot_of = np.empty(n_pad, np.int32)
    for node in order:
        load, w, cnt = heapq.heappop(heap)
        win_of[node] = w
        slot_of[node] = cnt
        cnt += 1
        if cnt < P:
            heapq.heappush(heap, (load + degp[node], w, cnt))
    return win_of.astype(np.int64) * P + slot_of


def wrap_idx(idx):
    n = len(idx)
    blk = np.asarray(idx, np.int16).reshape(n // 16, 16).T
    return np.tile(blk, (8, 1))


def prep_graph(src, dst, cfg):
    c = cfg
    deg = np.bincount(dst, minlength=c.n_nodes)
    new_id = balance_windows(deg, c)
    node_at = np.full(c.n_pad, -1, np.int64)
    node_at[new_id] = np.arange(c.n_pad)

    src_n = new_id[src]
    dst_n = new_id[dst]
    order = np.argsort(dst_n, kind="stable")
    src_s, dst_s = src_n[order], dst_n[order]
    win_s = dst_s // P
    bounds = np.searchsorted(win_s, np.arange(c.nw + 1))

    def rot_rows(core, pos):
        return ((pos // P - core * c.wpc) % c.nw) * P + pos % P

    TA = TB = 0
    grp_masks = {}
    for core in range(c.n_cores):
        for wl in range(c.wpc):
            gidx = core * c.wpc + wl
            lo, hi = bounds[gidx], bounds[gidx + 1]
            rr = rot_rows(core, src_s[lo:hi])
            mA = rr < c.split
            grp_masks[(core, wl)] = (lo, hi, rr, mA)
            nA = int(mA.sum())
            nB = int((hi - lo) - nA)
            TA = max(TA, -(-nA // P))
            TB = max(TB, -(-nB // P))
    TA = max(TA, 1)
    TB = max(TB, 1)
    T = TA + TB

    nco, wpc = c.n_cores, c.wpc
    idx16 = np.zeros((nco, wpc, P, T * 8), np.int16)
    dmb = np.full((nco, wpc, P, T), -1000.0, np.float32)
    epos = np.full((nco, wpc, T * P), -1, np.int64)

    for core in range(nco):
        for wl in range(wpc):
            lo, hi, rr, mA = grp_masks[(core, wl)]
            iA = np.where(mA)[0]
            iB = np.where(~mA)[0]
            la = np.zeros(TA * P, np.int64)
            lb = np.zeros(TB * P, np.int64)
            la[: len(iA)] = rr[iA]
            lb[: len(iB)] = rr[iB] - c.split
            idx16[core, wl, :, : TA * 8] = wrap_idx(la)
            idx16[core, wl, :, TA * 8 :] = wrap_idx(lb)
            dv = np.full(T * P, -1000.0, np.float32)
            dv[: len(iA)] = (dst_s[lo:hi][iA] % P).astype(np.float32)
            dv[TA * P : TA * P + len(iB)] = (dst_s[lo:hi][iB] % P).astype(np.float32)
            dmb[core, wl] = dv.reshape(T, P).T
            ep = np.full(T * P, -1, np.int64)
            ep[: len(iA)] = lo + iA
            ep[TA * P : TA * P + len(iB)] = lo + iB
            epos[core, wl] = ep

    return dict(new_id=new_id, node_at=node_at, order=order,
                TA=TA, TB=TB, idx16=idx16, dmb=dmb, epos=epos)


def edge_w_tables(g, w_edge, cfg):
    """(nc, wpc, P, T) f32 edge weights in tile layout."""
    ws = np.asarray(w_edge)[g["order"]]
    ep = g["epos"]
    out = np.where(ep >= 0, ws[np.clip(ep, 0, None)], 0.0).astype(np.float32)
    n, w, TP = out.shape
    T = TP // P
    return out.reshape(n, w, T, P).transpose(0, 1, 3, 2).copy()


def pack_dmf(g, w_am, w_ph, cfg):
    """[nc, wpc, 128, 3T] bf16: [0:T) dmb, then per t: (w_am, w_ph)."""
    nco, wpc, _, T = g["dmb"].shape
    dmf = np.zeros((nco, wpc, P, 3 * T), np.float32)
    dmf[..., 0:T] = g["dmb"]
    dmf[..., T::2] = w_am
    dmf[..., T + 1 :: 2] = w_ph
    return dmf.astype(BF)


def prep_conv(W, al, ar, b, mean_heads, cfg):
    H, F = cfg.heads, cfg.fdim
    W = np.asarray(W, np.float64)
    al = np.asarray(al, np.float64)
    ar = np.asarray(ar, np.float64)
    b = np.asarray(b, np.float64)
    T_bd = np.zeros((H * F, H * F))
    for h in range(H):
        a = al[h]
        M = np.concatenate([a[:, None], np.eye(F)[:, : F - 1]], 1)
        Q, _ = np.linalg.qr(M)
        blk = np.concatenate([a[:, None], Q[:, 1:]], 1)
        T_bd[h * F : (h + 1) * F, h * F : (h + 1) * F] = blk
    Tinv = np.linalg.inv(T_bd)
    R_bd = np.zeros((H * F, H))
    for h in range(H):
        R_bd[h * F : (h + 1) * F, h] = ar[h]
    # f-major column order for the ft block: new col f*H + h <- old h*F + f.
    # (el for head h rides at f=0, c=h.)
    Wft = (W @ T_bd).reshape(-1, H, F).transpose(0, 2, 1).reshape(-1, H * F)
    out = dict(
        Wg=np.concatenate([Wft, W @ R_bd], 1).astype(BF),
    )
    Tinv_fm = Tinv.reshape(H, F, H * F).transpose(1, 0, 2).reshape(H * F, H * F)
    if mean_heads:
        Mm = np.zeros((H * F, F))
        for h in range(H):
            Mm[h * F : (h + 1) * F] = np.eye(F) / H
        out["Tinv"] = (Tinv_fm @ Mm).astype(BF)
        out["bcol"] = (b.reshape(H, F).mean(0))[:, None].astype(np.float32)
    else:
        out["Tinv"] = Tinv_fm.astype(BF)
        out["bcol"] = b[:, None].astype(np.float32)
    return out


def consts_np(cfg, T):
    iota3 = np.tile(np.arange(P, dtype=np.float32)[None, :, None],
                    (P, 1, T)).astype(BF)
    ident = np.eye(P, dtype=np.float32).astype(BF)
    return dict(iota3=iota3, ident=ident)


def pack_meta(g, dmf):
    """[nc, wpc, 128, T*8 + 3T] int16: gather idxs then dmf bits."""
    return np.concatenate([g["idx16"], dmf.view(np.int16)], axis=-1)


def to_xT_tiled(x, g, cfg):
    """x (n_nodes, D) -> permuted transposed tiles (nw, D, 128) f32."""
    n_pad = cfg.n_pad
    D = x.shape[1]
    xp = np.zeros((n_pad, D), np.float32)
    real = g["node_at"] >= 0
    idx = g["node_at"][real]
    keep = idx < cfg.n_nodes
    xp[np.where(real)[0][keep]] = np.asarray(x, np.float32)[idx[keep]]
    return np.ascontiguousarray(xp.reshape(cfg.nw, P, D).transpose(0, 2, 1))


def rotate_flat_bf16(x_tiled, core, cfg):
    """(nw, D, 128) -> rotated, [D, nw*128] bf16 for `core`."""
    rot = np.roll(np.arange(cfg.nw), -core * cfg.wpc)
    r = x_tiled[rot]                       # (nw, D, 128)
    D = r.shape[1]
    return np.ascontiguousarray(
        r.transpose(1, 0, 2).reshape(D, cfg.nw * P)).astype(BF)


# ------------------------------------------------------------ layer program

def build_layer(cfg, TA, TB, last, dbg=()):
    """dbg: set of feature-disable flags for HW bisection:
    'nogather' - replace dma_gathers with memset
    'split_gather' - issue gathers in 4-tile batches
    'notrans' - skip PE transposes/ST path, memset er
    'noepi_t' - skip epilogue transpose, use sca directly (wrong results)
    """
    c = cfg
    T = TA + TB
    HF = c.heads * c.fdim            # 128
    OC = c.fdim if last else HF
    ODT = F32 if last else BF16
    nc = bacc.Bacc("TRN2", target_bir_lowering=False, debug=False)

    def dram_in(name, shape, dt=BF16):
        return nc.dram_tensor(name, list(shape), dt, kind="ExternalInput")

    xT2 = dram_in("xT2", (2, c.in_dim, c.nw * P))
    Wg = {s: dram_in(f"Wg_{s}", (c.in_dim, HF + c.heads)) for s in ("am", "ph")}
    Tinv = {s: dram_in(f"Tinv_{s}", (HF, OC)) for s in ("am", "ph")}
    bcol = {s: dram_in(f"bcol_{s}", (OC, 1), F32) for s in ("am", "ph")}
    iota3_d = dram_in("iota3", (P, P, T))
    ident_d = dram_in("ident", (P, P))
    meta_d = dram_in("meta", (c.wpc, P, T * 8 + 3 * T), I16)

    out_d = nc.dram_tensor("out2", [c.wpc, OC, 2, P], ODT, kind="ExternalOutput")
    stacks = ("am", "ph")

    NA = c.split                 # rows in table A
    NB = c.n_pad - c.split
    CH = 8                       # windows per phase-A chunk
    SC = 6                       # transposes per PSUM chunk (T=18 -> 3 chunks)
    n_chunk = -(-T // SC)

    with tile.TileContext(nc) as tc:
        with (
            tc.tile_pool(name="dram", bufs=1, space="DRAM") as dpool,
            tc.tile_pool(name="const", bufs=1) as cpool,
        ):
            gtabA = dpool.tile([NA, 2 * HF], BF16, name="gtabA", tag="gtabA")
            gtabB = dpool.tile([NB, 2 * HF], BF16, name="gtabB", tag="gtabB")

            ct = {}
            t = cpool.tile([P, P, T], BF16, name="ct_iota3")
            nc.sync.dma_start(t[:], iota3_d[:])
            ct["iota3"] = t
            t = cpool.tile([P, P], BF16, name="ct_ident")
            nc.sync.dma_start(t[:], ident_d[:])
            ct["ident"] = t
            for s in stacks:
                for nm, hd, dt in [("Wg", Wg[s], BF16), ("Tinv", Tinv[s], BF16),
                                   ("bcol", bcol[s], F32)]:
                    t = cpool.tile(list(hd.shape), dt, name=f"ct_{nm}_{s}")
                    nc.sync.dma_start(t[:], hd[:])
                    ct[(nm, s)] = t
            erw = cpool.tile([P, c.wpc, 2 * c.heads], BF16, name="erw")

            # ---------------- phase A ----------------
            with (
                tc.tile_pool(name="pa_x", bufs=3) as pax,
                tc.tile_pool(name="pa_g", bufs=3) as pag,
                tc.tile_pool(name="pa_ps", bufs=4, space="PSUM") as paps,
            ):
                for w0 in range(0, c.nw, CH):
                    xw = pax.tile([c.in_dim, 2, CH, P], BF16, name="xw", tag="x")
                    nc.sync.dma_start(
                        xw[:],
                        xT2[:, :, w0 * P : (w0 + CH) * P]
                        .rearrange("s d (w p) -> d s w p", p=P))
                    gsb = pag.tile([P, CH, 2 * HF], BF16, name="gsb", tag="gsb")
                    for k in range(CH):
                        g_ps = paps.tile([P, 2, HF + c.heads], F32,
                                         name="g_ps", tag="g")
                        for si, s in enumerate(stacks):
                            nc.tensor.matmul(g_ps[:, si, :], xw[:, si, k, :],
                                             ct[("Wg", s)][:],
                                             start=True, stop=True)
                        # row col = f*8 + s*4 + h (c-packed per f)
                        dst = gsb[:, k, :].rearrange("p (f s h) -> p s f h",
                                                     s=2, h=c.heads)
                        src = g_ps[:, :, 0:HF].rearrange("p s (f h) -> p s f h",
                                                         h=c.heads)
                        if (w0 + k) % 2 == 0:
                            nc.scalar.copy(dst, src)
                        else:
                            nc.vector.tensor_copy(dst, src)
                        if w0 + k < c.wpc:
                            for si in range(2):
                                nc.vector.tensor_copy(
                                    erw[:, w0 + k,
                                        si * c.heads : (si + 1) * c.heads],
                                    g_ps[:, si, HF : HF + c.heads])
                    if w0 * P < NA:
                        dst_ap = gtabA[w0 * P : (w0 + CH) * P, :]
                    else:
                        dst_ap = gtabB[w0 * P - NA : (w0 + CH) * P - NA, :]
                    nc.sync.dma_start(
                        dst_ap.rearrange("(w p) d -> p w d", p=P), gsb[:])

            # ---------------- phase B (software-pipelined) ----------------
            with (
                tc.tile_pool(name="pb_meta", bufs=4) as pbm,
                tc.tile_pool(name="pb_z", bufs=4) as pbz,
                tc.tile_pool(name="pb_s", bufs=4) as pbs,
                tc.tile_pool(name="pb_st", bufs=3) as pbst,
                tc.tile_pool(name="pb_small", bufs=3) as pbsm,
                tc.tile_pool(name="pb_epi", bufs=2) as pbepi,
                tc.tile_pool(name="ps_t", bufs=2, space="PSUM") as pst,
                tc.tile_pool(name="ps_er", bufs=2, space="PSUM") as pser,
                tc.tile_pool(name="ps_acc", bufs=2, space="PSUM") as psacc,
                tc.tile_pool(name="ps_epi", bufs=1, space="PSUM") as psepi,
                tc.tile_pool(name="ps_den", bufs=1, space="PSUM") as psden,
            ):
                regs = {}

                def _reg(n):
                    if n not in regs:
                        regs[n] = nc.gpsimd.to_reg(n)
                    return regs[n]

                st_meta = {}
                st_z = {}
                st_s = {}
                st_er = {}
                st_pz = {}
                st_acc = {}

                def s0(wl):
                    """meta load + gathers."""
                    meta_t = pbm.tile([P, T * 8 + 3 * T], I16, name="meta",
                                      tag="meta")
                    nc.sync.dma_start(meta_t[:], meta_d[wl])
                    Z = pbz.tile([P, T, 2 * HF], BF16, name="Z", tag="Z")
                    if 'nogather' in dbg:
                        nc.vector.memset(Z[:], 0.01)
                    else:
                        # HW SWDGE wedges above ~1024 idxs per call; batch.
                        gb = c.gbatch
                        for grp, tt, base, tab in (("A", TA, 0, gtabA),
                                                   ("B", TB, TA, gtabB)):
                            for off in range(0, tt, gb):
                                nb = min(gb, tt - off)
                                nc.gpsimd.dma_gather(
                                    out_ap=Z[:, base + off : base + off + nb, :],
                                    in_ap=tab[:],
                                    idxs_ap=meta_t[:, (base + off) * 8
                                                   : (base + off + nb) * 8],
                                    num_idxs=nb * P, num_idxs_reg=_reg(nb * P),
                                    elem_size=2 * HF)
                    st_meta[wl] = meta_t
                    st_z[wl] = Z

                def s1(wl):
                    """S one-hot + ST transpose + er matmuls."""
                    meta_t = st_meta[wl]
                    dmf_t = meta_t[:, T * 8 : T * 8 + 3 * T].bitcast(BF16)
                    # S one-hot, t-last layout [p, node, t] (packed for 2x)
                    S = pbs.tile([P, P, T], BF16, name="S", tag="S")
                    nc.vector.tensor_tensor(
                        out=S[:],
                        in0=ct["iota3"][:],
                        in1=dmf_t[:, 0:T].unsqueeze(1).broadcast_to([P, P, T]),
                        op=mybir.AluOpType.is_equal)
                    er_ps = pser.tile([P, T, 2 * c.heads], F32, name="er_ps",
                                      tag="erps")
                    if 'notrans' in dbg:
                        nc.vector.memset(er_ps[:], 0.0)
                    else:
                        STb = pbst.tile([P, T, P], BF16, name="ST", tag="ST")
                        for ch in range(n_chunk):
                            t0 = ch * SC
                            t1 = min(T, t0 + SC)
                            st_ps = pst.tile([P, SC, P], BF16, name="st_ps",
                                             tag="st")
                            for t in range(t0, t1):
                                nc.tensor.transpose(
                                    st_ps[:, t - t0, :], S[:, :, t],
                                    ct["ident"][:])
                            nc.scalar.copy(STb[:, t0:t1, :],
                                           st_ps[:, 0 : t1 - t0, :])
                        for t in range(T):
                            nc.tensor.matmul(er_ps[:, t, :], STb[:, t, :],
                                             erw[:, wl, :],
                                             start=True, stop=True)
                    st_s[wl] = S
                    st_er[wl] = er_ps

                def s2(wl):
                    """tb = el + er ; lrelu ; p = exp ; cb = p*w ; Z *= cb.
                    Z row layout: col = f*8 + s*4 + h (c packed last)."""
                    Z = st_z[wl]
                    er_ps = st_er[wl]
                    meta_t = st_meta[wl]
                    dmf_t = meta_t[:, T * 8 : T * 8 + 3 * T].bitcast(BF16)
                    tb = pbsm.tile([P, T, 2 * c.heads], F32, name="tb", tag="tb")
                    Zc = Z[:].rearrange("p t (f c) -> p t f c", c=2 * c.heads)
                    nc.vector.tensor_tensor(
                        out=tb[:], in0=Zc[:, :, 0:1, :].squeeze(2),
                        in1=er_ps[:], op=mybir.AluOpType.add)
                    nc.vector.scalar_tensor_tensor(
                        out=tb[:], in0=tb[:], scalar=NEG, in1=tb[:],
                        op0=mybir.AluOpType.mult, op1=mybir.AluOpType.max)
                    pz = pbsm.tile([P, T, 2 * c.heads], BF16, name="pz",
                                   tag="pz")
                    nc.scalar.activation(pz[:], tb[:],
                                         mybir.ActivationFunctionType.Exp)
                    cb = pbsm.tile([P, T, 2 * c.heads], BF16, name="cb",
                                   tag="cb")
                    nc.vector.tensor_tensor(
                        out=cb[:].rearrange("p t (s h) -> p t s h", s=2),
                        in0=pz[:].rearrange("p t (s h) -> p t s h", s=2),
                        in1=dmf_t[:, T : 3 * T]
                            .rearrange("p (t s) -> p t s", s=2)
                            .unsqueeze(3).broadcast_to([P, T, 2, c.heads]),
                        op=mybir.AluOpType.mult)
                    nc.vector.tensor_tensor(
                        out=Zc, in0=Zc,
                        in1=cb[:].unsqueeze(2)
                            .broadcast_to([P, T, c.fdim, 2 * c.heads]),
                        op=mybir.AluOpType.mult)
                    st_pz[wl] = pz

                def s3(wl):
                    """scatter matmuls."""
                    Z = st_z[wl]
                    S = st_s[wl]
                    pz = st_pz[wl]
                    num_t = psacc.tile([P, 2 * HF], F32, name="num", tag="num")
                    den_t = psden.tile([P, 2 * c.heads], F32, name="den",
                                       tag="den")
                    for t in range(T):
                        nc.tensor.matmul(num_t[:], S[:, :, t], Z[:, t, :],
                                         start=(t == 0), stop=(t == T - 1))
                        nc.tensor.matmul(den_t[:], S[:, :, t], pz[:, t, :],
                                         start=(t == 0), stop=(t == T - 1))
                    st_acc[wl] = (num_t, den_t)

                def s4(wl):
                    """epilogue + output write."""
                    num_t, den_t = st_acc.pop(wl)
                    num_ps = num_t[:]
                    den_ps = den_t[:]
                    denm = pbepi.tile([P, 2 * c.heads], F32, name="denm",
                                      tag="denm")
                    nc.vector.tensor_scalar(
                        out=denm[:], in0=den_ps, scalar1=1e-9, scalar2=None,
                        op0=mybir.AluOpType.max)
                    rec = pbepi.tile([P, 2 * c.heads], F32, name="rec",
                                     tag="rec")
                    nc.vector.reciprocal(rec[:], denm[:])
                    sca = pbepi.tile([P, 2, HF], BF16, name="sca", tag="sca")
                    numv = num_ps.rearrange("p (f s h) -> p f s h",
                                            s=2, h=c.heads)
                    recv = rec[:].rearrange("p (s h) -> p s h", s=2)
                    h2 = pbepi.tile([OC, 2, P], ODT, name="h2", tag="h2")
                    for si, s in enumerate(stacks):
                        nc.vector.tensor_tensor(
                            out=sca[:, si, :].rearrange("p (f h) -> p f h",
                                                        h=c.heads),
                            in0=numv[:, :, si, :],
                            in1=recv[:, si : si + 1, :]
                                .broadcast_to([P, c.fdim, c.heads]),
                            op=mybir.AluOpType.mult)
                        scT = pbepi.tile([P, P], BF16, name="scT", tag="scT")
                        if 'noepi_t' in dbg:
                            nc.scalar.copy(scT[:], sca[:, si, :])
                        else:
                            tr_ps = pst.tile([P, SC, P], BF16, name="st_ps",
                                             tag="st")
                            nc.tensor.transpose(
                                tr_ps[:, 0, :], sca[:, si, :],
                                ct["ident"][:])
                            nc.scalar.copy(scT[:], tr_ps[:, 0, :])
                        h_ps = psepi.tile([OC, P], F32, name="h_ps", tag="hps")
                        nc.tensor.matmul(h_ps[:], ct[("Tinv", s)][:], scT[:],
                                         start=True, stop=True)
                        nc.scalar.activation(
                            h2[:, si, :], h_ps[:],
                            (mybir.ActivationFunctionType.Identity if last
                             else mybir.ActivationFunctionType.Relu),
                            bias=ct[("bcol", s)][:], scale=1.0)
                    nc.sync.dma_start(out_d[wl], h2[:])
                    st_meta.pop(wl)
                    st_z.pop(wl)
                    st_s.pop(wl)
                    st_er.pop(wl)
                    st_pz.pop(wl)

                s0(0)
                for i in range(c.wpc + 2):
                    if i + 1 < c.wpc:
                        s0(i + 1)
                    if i < c.wpc:
                        s1(i)
                    if 1 <= i <= c.wpc:
                        s2(i - 1)
                        s3(i - 1)
                    if i >= 2:
                        s4(i - 2)
    nc.compile()
    return nc


# ------------------------------------------------------------ full pipeline

def make_in_maps(cfg, g, cc, xTf_am, xTf_ph, meta, conv_am, conv_ph):
    """xTf_*: per-core list of [D, nw*128] bf16."""
    maps = []
    for core in range(cfg.n_cores):
        maps.append(dict(
            xT2=np.ascontiguousarray(
                np.stack([xTf_am[core], xTf_ph[core]], 0)),
            Wg_am=conv_am["Wg"], Tinv_am=conv_am["Tinv"], bcol_am=conv_am["bcol"],
            Wg_ph=conv_ph["Wg"], Tinv_ph=conv_ph["Tinv"], bcol_ph=conv_ph["bcol"],
            iota3=cc["iota3"], ident=cc["ident"],
            meta=meta[core],
        ))
    return maps


def run_pipeline(inputs, cfg, runner):
    g = prep_graph(np.asarray(inputs["src"]), np.asarray(inputs["dst"]), cfg)
    cc = consts_np(cfg, g["TA"] + g["TB"])
    w_am = edge_w_tables(g, inputs["am_exist"], cfg)
    w_ph = edge_w_tables(g, inputs["exist"], cfg)
    dmf = pack_dmf(g, w_am, w_ph, cfg)
    meta = pack_meta(g, dmf)

    conv0a = prep_conv(inputs["W0a"], inputs["al0a"], inputs["ar0a"],
                       inputs["b0a"], False, cfg)
    conv0p = prep_conv(inputs["W0p"], inputs["al0p"], inputs["ar0p"],
                       inputs["b0p"], False, cfg)
    conv1a = prep_conv(inputs["W1a"], inputs["al1a"], inputs["ar1a"],
                       inputs["b1a"], True, cfg)
    conv1p = prep_conv(inputs["W1p"], inputs["al1p"], inputs["ar1p"],
                       inputs["b1p"], True, cfg)

    xT_am = to_xT_tiled(np.asarray(inputs["x_am"]), g, cfg)
    xT_ph = to_xT_tiled(np.asarray(inputs["x_ph"]), g, cfg)
    xTf_am = [rotate_flat_bf16(xT_am, cr, cfg) for cr in range(cfg.n_cores)]
    xTf_ph = [rotate_flat_bf16(xT_ph, cr, cfg) for cr in range(cfg.n_cores)]

    nc0 = build_layer(cfg, g["TA"], g["TB"], last=False)
    maps0 = make_in_maps(cfg, g, cc, xTf_am, xTf_ph, meta, conv0a, conv0p)
    outs0 = runner(nc0, maps0)

    # out2: per core (wpc, 128, 2, 128) bf16 feat-major tiles
    h2 = np.concatenate([np.asarray(o["out2"]) for o in outs0], 0)
    hT_am = np.ascontiguousarray(h2[:, :, 0, :])   # (nw, D, 128)
    hT_ph = np.ascontiguousarray(h2[:, :, 1, :])
    hf_am = [rotate_flat_bf16(hT_am, cr, cfg) for cr in range(cfg.n_cores)]
    hf_ph = [rotate_flat_bf16(hT_ph, cr, cfg) for cr in range(cfg.n_cores)]

    nc1 = build_layer(cfg, g["TA"], g["TB"], last=True)
    maps1 = make_in_maps(cfg, g, cc, hf_am, hf_ph, meta, conv1a, conv1p)
    outs1 = runner(nc1, maps1)

    o2 = np.concatenate([np.asarray(o["out2"], np.float32) for o in outs1], 0)
    # (nw, 32, 2, 128) -> (n_pad, 32)
    oam = o2[:, :, 0, :].transpose(0, 2, 1).reshape(cfg.n_pad, cfg.fdim)
    oph = o2[:, :, 1, :].transpose(0, 2, 1).reshape(cfg.n_pad, cfg.fdim)
    nid = g["new_id"][: cfg.n_nodes]
    return oam[nid], oph[nid]


# ------------------------------------------------------------ timed runner

def run_layer_timed(nc, in_maps, n_cores, repeats=3):
    import time as _time
    import jax
    from jax.sharding import Mesh, PartitionSpec, NamedSharding
    from concourse import bass2jax
    from jax.experimental.shard_map import shard_map

    bass2jax.install_neuronx_cc_hook()
    part_name = (nc.partition_id_tensor.name
                 if nc.partition_id_tensor is not None else None)
    in_names, out_names, out_avals, zero_outs = [], [], [], []
    for alloc in nc.m.functions[0].allocations:
        if not isinstance(alloc, mybir.MemoryLocationSet):
            continue
        name = alloc.memorylocations[0].name
        if alloc.kind == "ExternalInput":
            if name != part_name:
                in_names.append(name)
        elif alloc.kind == "ExternalOutput":
            out_names.append(name)
            shape = tuple(alloc.tensor_shape)
            dtype = mybir.dt.np(alloc.dtype)
            out_avals.append(jax.core.ShapedArray(shape, dtype))
            zero_outs.append(np.zeros(shape, dtype))
    n_params = len(in_names)
    all_in = list(in_names + out_names)
    if part_name is not None:
        all_in.append(part_name)

    def _body(*args):
        operands = list(args)
        if part_name is not None:
            operands.append(bass2jax.partition_id_tensor())
        outs = bass2jax._bass_exec_p.bind(
            *operands, out_avals=tuple(out_avals), in_names=tuple(all_in),
            out_names=tuple(out_names), lowering_input_output_aliases=(),
            sim_require_finite=True, sim_require_nnan=True, nc=nc)
        return tuple(outs)

    devices = jax.devices()[:n_cores]
    mesh = Mesh(np.asarray(devices), ("core",))
    spec = PartitionSpec("core")
    nin = n_params + len(out_names)
    f = jax.jit(shard_map(_body, mesh=mesh, in_specs=(spec,) * nin,
                          out_specs=(spec,) * len(out_names), check_rep=False))
    concat_in = [np.concatenate([np.asarray(m[nm]) for m in in_maps], 0)
                 for nm in in_names]
    concat_zeros = [np.zeros((n_cores * z.shape[0], *z.shape[1:]), z.dtype)
                    for z in zero_outs]
    sh = NamedSharding(mesh, spec)
    dev_in = [jax.device_put(a, sh) for a in concat_in]
    dev_zero = [jax.device_put(a, sh) for a in concat_zeros]
    outs = f(*dev_in, *dev_zero)
    jax.block_until_ready(outs)
    ts = []
    for _ in range(repeats):
        t0 = _time.perf_counter()
        o2 = f(*dev_in, *dev_zero)
        jax.block_until_ready(o2)
        ts.append(_time.perf_counter() - t0)
    res = []
    for cr in range(n_cores):
        res.append({nm: np.asarray(outs[i]).reshape(n_cores, *out_avals[i].shape)[cr]
                    for i, nm in enumerate(out_names)})
    return res, ts


def baseline_overhead(n_cores, repeats=5):
    nc = bacc.Bacc("TRN2", target_bir_lowering=False, debug=False)
    x = nc.dram_tensor("x", [P, P], F32, kind="ExternalInput")
    y = nc.dram_tensor("y", [P, P], F32, kind="ExternalOutput")
    with tile.TileContext(nc) as tc:
        with tc.tile_pool(name="p", bufs=1) as p:
            t = p.tile([P, P], F32)
            nc.sync.dma_start(t[:], x[:])
            nc.scalar.mul(t[:], t[:], 2.0)
            nc.sync.dma_start(y[:], t[:])
    nc.compile()
    maps = [{"x": np.zeros((P, P), np.float32)} for _ in range(n_cores)]
    _, ts = run_layer_timed(nc, maps, n_cores, repeats=repeats)
    return min(ts)


# ------------------------------------------------------------ kernel entry

_PERF = {"launch_info": []}


def _hw_runner(cfg, measure):
    from concourse.bass_utils import run_bass_kernel_spmd

    def run(nc, in_maps):
        if measure:
            res, ts = run_layer_timed(nc, in_maps, cfg.n_cores, repeats=10)
            _PERF["launch_info"].append(min(ts))
            return res
        res = run_bass_kernel_spmd(nc, in_maps,
                                   core_ids=list(range(cfg.n_cores)))
        return res.results
    return run


def kernel(**inputs):
    import os
    cfg = Cfg()
    measure = bool(int(os.environ.get("GAT_MEASURE", "0")))
    res_am, res_ph = run_pipeline(inputs, cfg, _hw_runner(cfg, measure))
    return res_am, res_ph
